# revision 36
# baseline (speedup 1.0000x reference)
"""Trainium2 Bass kernel for a 2-layer LSTM (B=256, T=512, I=64, H=256) + linear head.

Strategy (hardcoded, self-contained):
  - Data-parallel over batch across 8 NeuronCores (32 batch elems per core).
  - Per core, both LSTM layers run step-by-step in a feature-blocked layout:
      gate PSUM tile [128=(hblk4, b32), 256=(gate4, hh2, hl32)]
    produced by col-group-packed bf16 matmuls (tile_position=(0, 32*m)) that
    share the small transposed-state stationary hT [k, 32]. bf16 operands
    avoid the fp32 LOW/HIGH double-pumped matmul passes (2x PE stream time);
    PSUM accumulation stays fp32 and the elementwise cell math stays fp32.
  - Input projection x@Wih.T and all biases ride the same PSUM accumulation
    (augmented ones-row trick), so there is no separate projection pass.
  - The kernel is recurrence-latency-bound: the wall time is the per-step
    dependency cycle mm -> sigmoid -> c-update -> tanh -> h -> transpose ->
    mm, with the two layers' chains interleaving at a half-round phase
    offset on the shared engines. Cell math is structured to shorten that
    cycle:
      * one merged Sigmoid over all four gates (g-gate weights are
        pre-scaled by 2 host-side; tanh(x) = 2*sigmoid(2x) - 1),
      * cell state tracked as c' = c/2 so the candidate term is ONE fused
        scalar_tensor_tensor op (sig(i)*(sig(g2)-0.5)) and tanh(c) is read
        back with the activation's free input scale (tanh(2*c')),
      * sig(f)*c' runs on GPSIMD off the critical path,
      * tanh writes its result to a PSUM bank (ScalarE sits closer to PSUM
        than to SBUF; measured ~3% whole-kernel win).
  - The two output linear layers have no nonlinearity between them and are
    folded host-side into a single [256, 4] matmul + bias.
  - All weights ship as ONE packed DRAM blob -> one DMA -> one HWDGE queue
    semaphore, keeping per-instruction sync-wait counts within HW limits.
"""

import os

import numpy as np

B, T, I, H, O = 256, 512, 64, 256, 4
NCORES = 8
BS = B // NCORES  # 32

# reference gate order is (i, f, g, o); we reorder to (i, f, o, g) so that the
# sigmoid gates are contiguous (cols 0:192) and tanh(g) is cols 192:256.
GATE_PERM = [0, 1, 3, 2]

# weight blob column offsets (fp32 elements, [128, WB_COLS])
OFF_W0 = 0        # Whh0 perm  [128, 2*1024]
OFF_W1 = 2048     # Whh1 perm  [128, 2*1024]
OFF_WX1 = 4096    # Wih1 perm  [128, 2*1024]
OFF_WX0 = 6144    # Wih0 perm + bias row, rows 0:65, [65, 1024]
OFF_B1 = 7168     # bias1 row, row 0, [1, 1024]
OFF_WF = 8192     # folded head weight [128, 2*4]
OFF_BF = 8200     # folded head bias, row 0, [1, 4]
OFF_XT = 8224     # x transposed + ones row, rows 0:65, [65, t_steps*32]
def _wb_cols(t_steps):
    return OFF_XT + t_steps * BS

_CACHED = {}


def _perm_cols(Wt):
    """Permute gate columns of [K, 1024] (col j = gate_orig*256 + h) into
    col = m*256 + gate_new*64 + hh*32 + hl, where h = hh*128 + m*32 + hl."""
    K = Wt.shape[0]
    W = Wt.reshape(K, 4, 256)[:, GATE_PERM, :]      # [K, gate, h]
    W = W.reshape(K, 4, 2, 4, 32)                    # [K, gate, hh, m, hl]
    W = W.transpose(0, 3, 1, 2, 4)                   # [K, m, gate, hh, hl]
    return np.ascontiguousarray(W.reshape(K, 1024), dtype=np.float32)


def _build_bass(t_steps=T):
    import concourse.mybir as mybir
    import concourse.tile as tile
    from concourse import bacc
    from contextlib import ExitStack

    f32 = mybir.dt.float32
    bf16 = mybir.dt.bfloat16
    AF = mybir.ActivationFunctionType

    nc = bacc.Bacc("TRN2", target_bir_lowering=False)

    wb_cols = _wb_cols(t_steps)
    wb_d = nc.dram_tensor("wb", (128, wb_cols), bf16, kind="ExternalInput")
    y_d = nc.dram_tensor("y", (BS, O), f32, kind="ExternalOutput")

    with tile.TileContext(nc) as tc, ExitStack() as ctx:
        const = ctx.enter_context(tc.tile_pool(name="const", bufs=1))
        state = ctx.enter_context(tc.tile_pool(name="state", bufs=1))
        work = ctx.enter_context(tc.tile_pool(name="work", bufs=6))
        hts = ctx.enter_context(tc.tile_pool(name="hts", bufs=4))
        psum = ctx.enter_context(tc.tile_pool(name="psum", bufs=2, space="PSUM"))

        wb = const.tile([128, wb_cols], bf16)
        nc.sync.dma_start(wb[:], wb_d[:])

        def xt_ap(t):
            return wb[0:65, OFF_XT + BS * t : OFF_XT + BS * t + BS]

        def w0_ap(kc, m):
            return wb[:, OFF_W0 + 1024 * kc + 256 * m : OFF_W0 + 1024 * kc + 256 * m + 256]

        def w1_ap(kc, m):
            return wb[:, OFF_W1 + 1024 * kc + 256 * m : OFF_W1 + 1024 * kc + 256 * m + 256]

        def wx1_ap(kc, m):
            return wb[:, OFF_WX1 + 1024 * kc + 256 * m : OFF_WX1 + 1024 * kc + 256 * m + 256]

        def wx0_ap(m):
            return wb[0:65, OFF_WX0 + 256 * m : OFF_WX0 + 256 * m + 256]

        def b1_ap(m):
            return wb[0:1, OFF_B1 + 256 * m : OFF_B1 + 256 * m + 256]

        c_psum = os.environ.get("LSTM_C_PSUM", "0") != "0"
        if c_psum:
            statep = ctx.enter_context(tc.tile_pool(name="statep", bufs=1, space="PSUM"))
            c0 = statep.tile([128, 64], f32, tag="c0")
            c1 = statep.tile([128, 64], f32, tag="c1")
        else:
            c0 = state.tile([128, 64], f32)
            c1 = state.tile([128, 64], f32)
        nc.vector.memset(c0[:], 0.0)
        nc.vector.memset(c1[:], 0.0)
        hT0 = hts.tile([128, 64], bf16, tag="ht0")
        hT1 = hts.tile([128, 64], bf16, tag="ht1")
        nc.vector.memset(hT0[:], 0.0)
        nc.vector.memset(hT1[:], 0.0)
        ones_t = const.tile([1, BS], bf16)
        nc.vector.memset(ones_t[:], 1.0)
        ones_ap = ones_t[:]

        from concourse.alu_op_type import AluOpType

        n_junk = int(os.environ.get("LSTM_JUNK", "0"))
        junkp = None
        if n_junk:
            junkp = ctx.enter_context(tc.tile_pool(name="junkp", bufs=2, space="PSUM"))

        def mms0(t, hT0_prev):
            g = psum.tile([128, 256], f32, tag="g0")
            for m in range(4):
                nc.tensor.matmul(
                    g[32 * m : 32 * m + 32, :], xt_ap(t), wx0_ap(m),
                    start=True, stop=False, tile_position=(0, 32 * m), skip_group_check=True,
                )
            for kc in range(2):
                for m in range(4):
                    nc.tensor.matmul(
                        g[32 * m : 32 * m + 32, :],
                        hT0_prev[:, 32 * kc : 32 * kc + 32], w0_ap(kc, m),
                        start=False, stop=(kc == 1), tile_position=(0, 32 * m), skip_group_check=True,
                    )
            return g

        def mms1(hT0_t, hT1_prev):
            g = psum.tile([128, 256], f32, tag="g1")
            for m in range(4):
                nc.tensor.matmul(
                    g[32 * m : 32 * m + 32, :], ones_ap, b1_ap(m),
                    start=True, stop=False, tile_position=(0, 32 * m), skip_group_check=True,
                )
            for src, w_ap in ((hT0_t, wx1_ap), (hT1_prev, w1_ap)):
                last_src = w_ap is w1_ap
                for kc in range(2):
                    for m in range(4):
                        nc.tensor.matmul(
                            g[32 * m : 32 * m + 32, :],
                            src[:, 32 * kc : 32 * kc + 32], w_ap(kc, m),
                            start=False, stop=(last_src and kc == 1),
                            tile_position=(0, 32 * m), skip_group_check=True,
                        )
            return g

        def junk_mms(n):
            # keep-warm matmuls: the PE HAM clock-gate only releases the
            # 2.4 GHz clock when the array looks busy; the real mm stream
            # alone leaves it at 1.2 GHz with micro-idles. Short N=128
            # streams into a scratch psum bank bound the queue-blocking cost.
            for _ in range(n):
                jt = junkp.tile([32, 128], f32, tag="junk")
                nc.tensor.matmul(jt[:], wb[:, 0:32], wb[:, 0:128],
                                 start=True, stop=True)

        def sig(g, tagsuf):
            # g-gate pre-activations arrive pre-scaled by 2 (host-side weight
            # prescale), so ONE Sigmoid covers all four gates:
            # tanh(x) = 2*sigmoid(2x) - 1.
            sg = work.tile([128, 256], f32, tag="sg" + tagsuf)
            nc.scalar.activation(sg[:], g[:], AF.Sigmoid)
            return sg

        def cphase(sg, c, tagsuf):
            # state is tracked as c' = c/2:
            #   tanh(g-pre) = 2*sig(g2) - 1, so
            #   m1' = sig(i)*tanh(g)/2 = sig(i)*(sig(g2) - 0.5)   [one fused STT]
            #   c'  = sig(f)*c' + m1'
            m1 = work.tile([128, 64], f32, tag="m1" + tagsuf)
            nc.vector.scalar_tensor_tensor(
                m1[:], sg[:, 192:256], 0.5, sg[:, 0:64],
                AluOpType.subtract, AluOpType.mult)
            nc.gpsimd.tensor_mul(c[:], sg[:, 64:128], c[:])
            nc.vector.tensor_add(c[:], c[:], m1[:])

        th_psum = os.environ.get("LSTM_TH_PSUM", "1") != "0"
        thp = None
        if th_psum:
            thp = ctx.enter_context(tc.tile_pool(name="thp", bufs=1, space="PSUM"))

        def tanph(c, tagsuf):
            # tanh(c) = tanh(2*c') via the free input affine
            pool = thp if th_psum else work
            th = pool.tile([128, 64], f32, tag="th" + tagsuf)
            nc.scalar.activation(th[:], c[:], AF.Tanh, scale=2.0)
            return th

        split_h = os.environ.get("LSTM_SPLIT_H", "0") != "0"

        def hphase(sg, th, tagsuf):
            # split h = sig(o)*th and the 32x32-block transpose into column
            # halves: the next step's kc0 matmul only needs hT[:, 0:32], so it
            # can start while the second half is still on the DVE.
            h = work.tile([128, 64], bf16, tag="h" + tagsuf)
            hT = hts.tile([128, 64], bf16, tag="ht" + tagsuf)
            if split_h:
                for half in range(2):
                    s = slice(32 * half, 32 * half + 32)
                    nc.vector.tensor_mul(h[:, s], sg[:, 128 + 32 * half : 160 + 32 * half], th[:, s])
                    nc.vector.transpose(hT[:, s], h[:, s])
            else:
                nc.vector.tensor_mul(h[:], sg[:, 128:192], th[:])
                nc.vector.transpose(hT[:], h[:])
            return hT

        # rounds: round t runs layer0 step t (A) and layer1 step t-1 (B).
        # In the half-offset steady state the per-engine readiness order is
        # full-A-chain then full-B-chain, so emit whole chains sequentially —
        # interleaving phases head-of-line-blocks the FIFO engine queues.
        hT0_prev, hT1_prev = hT0, hT1
        # Emission anchors the B chain a quarter-round behind A: queueing
        # sigB right after sigA on the ACT FIFO phase-locks the chains away
        # from the half-offset fixed point where B's tanh/transposes collide
        # with A's sigma/STT.
        # The two chains settle into a half-round phase offset; emitting each
        # chain's ops as a contiguous block in A-then-B order matches the
        # per-engine readiness order at that fixed point.
        for t in range(t_steps + 1):
            gA = mms0(t, hT0_prev) if t < t_steps else None
            if n_junk:
                junk_mms(n_junk // 2)
            gB = mms1(hT0_prev, hT1_prev) if t >= 1 else None
            if n_junk:
                junk_mms(n_junk - n_junk // 2)
            if gA is not None:
                sgA = sig(gA, "0")
                cphase(sgA, c0, "0")
                thA = tanph(c0, "0")
                hT0_new = hphase(sgA, thA, "0")
            if gB is not None:
                sgB = sig(gB, "1")
                cphase(sgB, c1, "1")
                thB = tanph(c1, "1")
                hT1_prev = hphase(sgB, thB, "1")
            if gA is not None:
                hT0_prev = hT0_new
        hT1 = hT1_prev

        yp = psum.tile([BS, O], f32, tag="yh", bufs=1)
        nc.tensor.matmul(yp[:], ones_ap, wb[0:1, OFF_BF : OFF_BF + O], start=True, stop=False)
        nc.tensor.matmul(yp[:], hT1[:, 0:32], wb[:, OFF_WF : OFF_WF + O], start=False, stop=False)
        nc.tensor.matmul(yp[:], hT1[:, 32:64], wb[:, OFF_WF + O : OFF_WF + 2 * O], start=False, stop=True)
        y_sb = work.tile([BS, O], f32, tag="y")
        nc.vector.tensor_copy(y_sb[:], yp[:])
        nc.sync.dma_start(y_d[:], y_sb[:])

    return nc


def _prep_inputs(x, Wih0, Whh0, bih0, bhh0, Wih1, Whh1, bih1, bhh1, W1, b1, W2, b2,
                 t_steps=T):
    x = np.asarray(x, dtype=np.float32)[:, :t_steps, :]

    def g2(Wt):
        # prescale the g-gate columns (original order i,f,g,o => cols 512:768)
        # by 2 so the kernel can compute tanh(x) as 2*sigmoid(2x)-1 with a
        # single Sigmoid activation over all four gates.
        Wt = np.asarray(Wt, np.float32).copy()
        Wt[..., 512:768] *= 2.0
        return Wt

    wb = np.zeros((128, _wb_cols(t_steps)), np.float32)
    wb[:, OFF_W0 : OFF_W0 + 2048] = _perm_cols(
        g2(np.asarray(Whh0, np.float32).T)).reshape(2, 128, 1024).transpose(1, 0, 2).reshape(128, 2048)
    wb[:, OFF_W1 : OFF_W1 + 2048] = _perm_cols(
        g2(np.asarray(Whh1, np.float32).T)).reshape(2, 128, 1024).transpose(1, 0, 2).reshape(128, 2048)
    wb[:, OFF_WX1 : OFF_WX1 + 2048] = _perm_cols(
        g2(np.asarray(Wih1, np.float32).T)).reshape(2, 128, 1024).transpose(1, 0, 2).reshape(128, 2048)
    wb[0:64, OFF_WX0 : OFF_WX0 + 1024] = _perm_cols(g2(np.asarray(Wih0, np.float32).T))
    wb[64, OFF_WX0 : OFF_WX0 + 1024] = _perm_cols(
        g2((np.asarray(bih0, np.float32) + np.asarray(bhh0, np.float32))[None, :]))[0]
    wb[0, OFF_B1 : OFF_B1 + 1024] = _perm_cols(
        g2((np.asarray(bih1, np.float32) + np.asarray(bhh1, np.float32))[None, :]))[0]
    Wf = (np.asarray(W1, np.float32).T @ np.asarray(W2, np.float32).T).astype(np.float32)
    wb[:, OFF_WF : OFF_WF + 2 * O] = Wf.reshape(2, 128, O).transpose(1, 0, 2).reshape(128, 2 * O)
    wb[0, OFF_BF : OFF_BF + O] = (
        np.asarray(b1, np.float32) @ np.asarray(W2, np.float32).T + np.asarray(b2, np.float32))

    import ml_dtypes

    in_maps = []
    for c in range(NCORES):
        xc = x[c * BS : (c + 1) * BS]                       # [BS, t, I]
        xt = xc.transpose(2, 1, 0).reshape(I, t_steps * BS) # [I, t*BS]
        wbc = wb.copy()
        wbc[0:64, OFF_XT:] = xt
        wbc[64, OFF_XT:] = 1.0
        in_maps.append(dict(wb=wbc.astype(ml_dtypes.bfloat16)))
    return in_maps


def run(t_steps=T, trace=False, **inputs):
    from concourse.bass_utils import run_bass_kernel_spmd

    key = t_steps
    if key not in _CACHED:
        nc_new = _build_bass(t_steps)
        # finalize BEFORE handing to the PJRT path: the bass_exec lowering
        # otherwise finalizes with the partition-id register preamble in a
        # state that miscompiles (walrus "Reg has not been allocated yet")
        nc_new.finalize()
        _CACHED[key] = nc_new
    nc = _CACHED[key]
    in_maps = _prep_inputs(**inputs, t_steps=t_steps)
    res = None
    for attempt in range(4):
        try:
            res = run_bass_kernel_spmd(nc, in_maps, core_ids=list(range(NCORES)),
                                       trace=trace)
            break
        except Exception as e:  # flaky parallel-birverifier race in neuronx-cc
            if attempt == 3:
                raise
            print(f"run attempt {attempt} failed ({type(e).__name__}); retrying")
    assert res is not None
    y = np.concatenate([r["y"] for r in res.results], axis=0)
    return y, res


def kernel(**inputs):
    y, _ = run(t_steps=T, trace=False, **inputs)
    return y



# revision 37
# speedup vs baseline: 1.0004x; 1.0004x over previous
"""Trainium2 Bass kernel for a 2-layer LSTM (B=256, T=512, I=64, H=256) + linear head.

Strategy (hardcoded, self-contained):
  - Data-parallel over batch across 8 NeuronCores (32 batch elems per core).
  - Per core, both LSTM layers run step-by-step in a feature-blocked layout:
      gate PSUM tile [128=(hblk4, b32), 256=(gate4, hh2, hl32)]
    produced by col-group-packed bf16 matmuls (tile_position=(0, 32*m)) that
    share the small transposed-state stationary hT [k, 32]. bf16 operands
    avoid the fp32 LOW/HIGH double-pumped matmul passes (2x PE stream time);
    PSUM accumulation stays fp32 and the elementwise cell math stays fp32.
  - Input projection x@Wih.T and all biases ride the same PSUM accumulation
    (augmented ones-row trick), so there is no separate projection pass.
  - The kernel is recurrence-latency-bound: the wall time is the per-step
    dependency cycle mm -> sigmoid -> c-update -> tanh -> h -> transpose ->
    mm, with the two layers' chains interleaving at a half-round phase
    offset on the shared engines. Cell math is structured to shorten that
    cycle:
      * one merged Sigmoid over all four gates (g-gate weights are
        pre-scaled by 2 host-side; tanh(x) = 2*sigmoid(2x) - 1),
      * cell state tracked as c' = c/2 so the candidate term is ONE fused
        scalar_tensor_tensor op (sig(i)*(sig(g2)-0.5)) and tanh(c) is read
        back with the activation's free input scale (tanh(2*c')),
      * sig(f)*c' runs on GPSIMD off the critical path,
      * tanh writes its result to a PSUM bank (ScalarE sits closer to PSUM
        than to SBUF; measured ~3% whole-kernel win).
  - The two output linear layers have no nonlinearity between them and are
    folded host-side into a single [256, 4] matmul + bias.
  - All weights ship as ONE packed DRAM blob -> one DMA -> one HWDGE queue
    semaphore, keeping per-instruction sync-wait counts within HW limits.
"""

import os

import numpy as np

B, T, I, H, O = 256, 512, 64, 256, 4
NCORES = 8
BS = B // NCORES  # 32

# reference gate order is (i, f, g, o); we reorder to (i, f, o, g) so that the
# sigmoid gates are contiguous (cols 0:192) and tanh(g) is cols 192:256.
GATE_PERM = [0, 1, 3, 2]

# weight blob column offsets (fp32 elements, [128, WB_COLS])
OFF_W0 = 0        # Whh0 perm  [128, 2*1024]
OFF_W1 = 2048     # Whh1 perm  [128, 2*1024]
OFF_WX1 = 4096    # Wih1 perm  [128, 2*1024]
OFF_WX0 = 6144    # Wih0 perm + bias row, rows 0:65, [65, 1024]
OFF_B1 = 7168     # bias1 row, row 0, [1, 1024]
OFF_WF = 8192     # folded head weight [128, 2*4]
OFF_BF = 8200     # folded head bias, row 0, [1, 4]
OFF_XT = 8224     # x transposed + ones row, rows 0:65, [65, t_steps*32]
def _wb_cols(t_steps):
    return OFF_XT + t_steps * BS

_CACHED = {}


def _perm_cols(Wt):
    """Permute gate columns of [K, 1024] (col j = gate_orig*256 + h) into
    col = m*256 + gate_new*64 + hh*32 + hl, where h = hh*128 + m*32 + hl."""
    K = Wt.shape[0]
    W = Wt.reshape(K, 4, 256)[:, GATE_PERM, :]      # [K, gate, h]
    W = W.reshape(K, 4, 2, 4, 32)                    # [K, gate, hh, m, hl]
    W = W.transpose(0, 3, 1, 2, 4)                   # [K, m, gate, hh, hl]
    return np.ascontiguousarray(W.reshape(K, 1024), dtype=np.float32)


def _build_bass(t_steps=T):
    import concourse.mybir as mybir
    import concourse.tile as tile
    from concourse import bacc
    from contextlib import ExitStack

    f32 = mybir.dt.float32
    bf16 = mybir.dt.bfloat16
    AF = mybir.ActivationFunctionType

    nc = bacc.Bacc("TRN2", target_bir_lowering=False)

    wb_cols = _wb_cols(t_steps)
    wb_d = nc.dram_tensor("wb", (128, wb_cols), bf16, kind="ExternalInput")
    y_d = nc.dram_tensor("y", (BS, O), f32, kind="ExternalOutput")

    with tile.TileContext(nc) as tc, ExitStack() as ctx:
        const = ctx.enter_context(tc.tile_pool(name="const", bufs=1))
        state = ctx.enter_context(tc.tile_pool(name="state", bufs=1))
        work = ctx.enter_context(tc.tile_pool(name="work", bufs=6))
        hts = ctx.enter_context(tc.tile_pool(name="hts", bufs=4))
        psum = ctx.enter_context(tc.tile_pool(
            name="psum", bufs=int(os.environ.get("LSTM_GBUFS", "2")), space="PSUM"))

        wb = const.tile([128, wb_cols], bf16)
        nc.sync.dma_start(wb[:], wb_d[:])

        def xt_ap(t):
            return wb[0:65, OFF_XT + BS * t : OFF_XT + BS * t + BS]

        def w0_ap(kc, m):
            return wb[:, OFF_W0 + 1024 * kc + 256 * m : OFF_W0 + 1024 * kc + 256 * m + 256]

        def w1_ap(kc, m):
            return wb[:, OFF_W1 + 1024 * kc + 256 * m : OFF_W1 + 1024 * kc + 256 * m + 256]

        def wx1_ap(kc, m):
            return wb[:, OFF_WX1 + 1024 * kc + 256 * m : OFF_WX1 + 1024 * kc + 256 * m + 256]

        def wx0_ap(m):
            return wb[0:65, OFF_WX0 + 256 * m : OFF_WX0 + 256 * m + 256]

        def b1_ap(m):
            return wb[0:1, OFF_B1 + 256 * m : OFF_B1 + 256 * m + 256]

        c_psum = os.environ.get("LSTM_C_PSUM", "0") != "0"
        if c_psum:
            statep = ctx.enter_context(tc.tile_pool(name="statep", bufs=1, space="PSUM"))
            c0 = statep.tile([128, 64], f32, tag="c0")
            c1 = statep.tile([128, 64], f32, tag="c1")
        else:
            c0 = state.tile([128, 64], f32)
            c1 = state.tile([128, 64], f32)
        nc.vector.memset(c0[:], 0.0)
        nc.vector.memset(c1[:], 0.0)
        hT0 = hts.tile([128, 64], bf16, tag="ht0")
        hT1 = hts.tile([128, 64], bf16, tag="ht1")
        nc.vector.memset(hT0[:], 0.0)
        nc.vector.memset(hT1[:], 0.0)
        ones_t = const.tile([1, BS], bf16)
        nc.vector.memset(ones_t[:], 1.0)
        ones_ap = ones_t[:]

        from concourse.alu_op_type import AluOpType

        n_junk = int(os.environ.get("LSTM_JUNK", "0"))
        junkp = None
        if n_junk:
            junkp = ctx.enter_context(tc.tile_pool(name="junkp", bufs=2, space="PSUM"))

        def mms0(t, hT0_prev):
            g = psum.tile([128, 256], f32, tag="g0")
            for m in range(4):
                nc.tensor.matmul(
                    g[32 * m : 32 * m + 32, :], xt_ap(t), wx0_ap(m),
                    start=True, stop=False, tile_position=(0, 32 * m), skip_group_check=True,
                )
            for kc in range(2):
                for m in range(4):
                    nc.tensor.matmul(
                        g[32 * m : 32 * m + 32, :],
                        hT0_prev[:, 32 * kc : 32 * kc + 32], w0_ap(kc, m),
                        start=False, stop=(kc == 1), tile_position=(0, 32 * m), skip_group_check=True,
                    )
            return g

        def mms1(hT0_t, hT1_prev):
            g = psum.tile([128, 256], f32, tag="g1")
            for m in range(4):
                nc.tensor.matmul(
                    g[32 * m : 32 * m + 32, :], ones_ap, b1_ap(m),
                    start=True, stop=False, tile_position=(0, 32 * m), skip_group_check=True,
                )
            for src, w_ap in ((hT0_t, wx1_ap), (hT1_prev, w1_ap)):
                last_src = w_ap is w1_ap
                for kc in range(2):
                    for m in range(4):
                        nc.tensor.matmul(
                            g[32 * m : 32 * m + 32, :],
                            src[:, 32 * kc : 32 * kc + 32], w_ap(kc, m),
                            start=False, stop=(last_src and kc == 1),
                            tile_position=(0, 32 * m), skip_group_check=True,
                        )
            return g

        def junk_mms(n):
            # keep-warm matmuls: the PE HAM clock-gate only releases the
            # 2.4 GHz clock when the array looks busy; the real mm stream
            # alone leaves it at 1.2 GHz with micro-idles. Short N=128
            # streams into a scratch psum bank bound the queue-blocking cost.
            for _ in range(n):
                jt = junkp.tile([32, 128], f32, tag="junk")
                nc.tensor.matmul(jt[:], wb[:, 0:32], wb[:, 0:128],
                                 start=True, stop=True)

        def sig(g, tagsuf):
            # g-gate pre-activations arrive pre-scaled by 2 (host-side weight
            # prescale), so ONE Sigmoid covers all four gates:
            # tanh(x) = 2*sigmoid(2x) - 1.
            sg = work.tile([128, 256], f32, tag="sg" + tagsuf)
            nc.scalar.activation(sg[:], g[:], AF.Sigmoid)
            return sg

        def cphase(sg, c, tagsuf):
            # state is tracked as c' = c/2:
            #   tanh(g-pre) = 2*sig(g2) - 1, so
            #   m1' = sig(i)*tanh(g)/2 = sig(i)*(sig(g2) - 0.5)   [one fused STT]
            #   c'  = sig(f)*c' + m1'
            m1 = work.tile([128, 64], f32, tag="m1" + tagsuf)
            nc.vector.scalar_tensor_tensor(
                m1[:], sg[:, 192:256], 0.5, sg[:, 0:64],
                AluOpType.subtract, AluOpType.mult)
            nc.gpsimd.tensor_mul(c[:], sg[:, 64:128], c[:])
            nc.vector.tensor_add(c[:], c[:], m1[:])

        th_psum = os.environ.get("LSTM_TH_PSUM", "1") != "0"
        thp = None
        if th_psum:
            thp = ctx.enter_context(tc.tile_pool(name="thp", bufs=1, space="PSUM"))

        def tanph(c, tagsuf):
            # tanh(c) = tanh(2*c') via the free input affine
            pool = thp if th_psum else work
            th = pool.tile([128, 64], f32, tag="th" + tagsuf)
            nc.scalar.activation(th[:], c[:], AF.Tanh, scale=2.0)
            return th

        split_h = os.environ.get("LSTM_SPLIT_H", "0") != "0"

        def hphase(sg, th, tagsuf):
            # split h = sig(o)*th and the 32x32-block transpose into column
            # halves: the next step's kc0 matmul only needs hT[:, 0:32], so it
            # can start while the second half is still on the DVE.
            h = work.tile([128, 64], bf16, tag="h" + tagsuf)
            hT = hts.tile([128, 64], bf16, tag="ht" + tagsuf)
            if split_h:
                for half in range(2):
                    s = slice(32 * half, 32 * half + 32)
                    nc.vector.tensor_mul(h[:, s], sg[:, 128 + 32 * half : 160 + 32 * half], th[:, s])
                    nc.vector.transpose(hT[:, s], h[:, s])
            else:
                nc.vector.tensor_mul(h[:], sg[:, 128:192], th[:])
                nc.vector.transpose(hT[:], h[:])
            return hT

        # rounds: round t runs layer0 step t (A) and layer1 step t-1 (B).
        # In the half-offset steady state the per-engine readiness order is
        # full-A-chain then full-B-chain, so emit whole chains sequentially —
        # interleaving phases head-of-line-blocks the FIFO engine queues.
        hT0_prev, hT1_prev = hT0, hT1
        # Emission anchors the B chain a quarter-round behind A: queueing
        # sigB right after sigA on the ACT FIFO phase-locks the chains away
        # from the half-offset fixed point where B's tanh/transposes collide
        # with A's sigma/STT.
        # The two chains settle into a half-round phase offset; emitting each
        # chain's ops as a contiguous block in A-then-B order matches the
        # per-engine readiness order at that fixed point.
        for t in range(t_steps + 1):
            gA = mms0(t, hT0_prev) if t < t_steps else None
            if n_junk:
                junk_mms(n_junk // 2)
            gB = mms1(hT0_prev, hT1_prev) if t >= 1 else None
            if n_junk:
                junk_mms(n_junk - n_junk // 2)
            if gA is not None:
                sgA = sig(gA, "0")
                cphase(sgA, c0, "0")
                thA = tanph(c0, "0")
                hT0_new = hphase(sgA, thA, "0")
            if gB is not None:
                sgB = sig(gB, "1")
                cphase(sgB, c1, "1")
                thB = tanph(c1, "1")
                hT1_prev = hphase(sgB, thB, "1")
            if gA is not None:
                hT0_prev = hT0_new
        hT1 = hT1_prev

        yp = psum.tile([BS, O], f32, tag="yh", bufs=1)
        nc.tensor.matmul(yp[:], ones_ap, wb[0:1, OFF_BF : OFF_BF + O], start=True, stop=False)
        nc.tensor.matmul(yp[:], hT1[:, 0:32], wb[:, OFF_WF : OFF_WF + O], start=False, stop=False)
        nc.tensor.matmul(yp[:], hT1[:, 32:64], wb[:, OFF_WF + O : OFF_WF + 2 * O], start=False, stop=True)
        y_sb = work.tile([BS, O], f32, tag="y")
        nc.vector.tensor_copy(y_sb[:], yp[:])
        nc.sync.dma_start(y_d[:], y_sb[:])

    return nc


def _prep_inputs(x, Wih0, Whh0, bih0, bhh0, Wih1, Whh1, bih1, bhh1, W1, b1, W2, b2,
                 t_steps=T):
    x = np.asarray(x, dtype=np.float32)[:, :t_steps, :]

    def g2(Wt):
        # prescale the g-gate columns (original order i,f,g,o => cols 512:768)
        # by 2 so the kernel can compute tanh(x) as 2*sigmoid(2x)-1 with a
        # single Sigmoid activation over all four gates.
        Wt = np.asarray(Wt, np.float32).copy()
        Wt[..., 512:768] *= 2.0
        return Wt

    wb = np.zeros((128, _wb_cols(t_steps)), np.float32)
    wb[:, OFF_W0 : OFF_W0 + 2048] = _perm_cols(
        g2(np.asarray(Whh0, np.float32).T)).reshape(2, 128, 1024).transpose(1, 0, 2).reshape(128, 2048)
    wb[:, OFF_W1 : OFF_W1 + 2048] = _perm_cols(
        g2(np.asarray(Whh1, np.float32).T)).reshape(2, 128, 1024).transpose(1, 0, 2).reshape(128, 2048)
    wb[:, OFF_WX1 : OFF_WX1 + 2048] = _perm_cols(
        g2(np.asarray(Wih1, np.float32).T)).reshape(2, 128, 1024).transpose(1, 0, 2).reshape(128, 2048)
    wb[0:64, OFF_WX0 : OFF_WX0 + 1024] = _perm_cols(g2(np.asarray(Wih0, np.float32).T))
    wb[64, OFF_WX0 : OFF_WX0 + 1024] = _perm_cols(
        g2((np.asarray(bih0, np.float32) + np.asarray(bhh0, np.float32))[None, :]))[0]
    wb[0, OFF_B1 : OFF_B1 + 1024] = _perm_cols(
        g2((np.asarray(bih1, np.float32) + np.asarray(bhh1, np.float32))[None, :]))[0]
    Wf = (np.asarray(W1, np.float32).T @ np.asarray(W2, np.float32).T).astype(np.float32)
    wb[:, OFF_WF : OFF_WF + 2 * O] = Wf.reshape(2, 128, O).transpose(1, 0, 2).reshape(128, 2 * O)
    wb[0, OFF_BF : OFF_BF + O] = (
        np.asarray(b1, np.float32) @ np.asarray(W2, np.float32).T + np.asarray(b2, np.float32))

    import ml_dtypes

    in_maps = []
    for c in range(NCORES):
        xc = x[c * BS : (c + 1) * BS]                       # [BS, t, I]
        xt = xc.transpose(2, 1, 0).reshape(I, t_steps * BS) # [I, t*BS]
        wbc = wb.copy()
        wbc[0:64, OFF_XT:] = xt
        wbc[64, OFF_XT:] = 1.0
        in_maps.append(dict(wb=wbc.astype(ml_dtypes.bfloat16)))
    return in_maps


def run(t_steps=T, trace=False, **inputs):
    from concourse.bass_utils import run_bass_kernel_spmd

    key = t_steps
    if key not in _CACHED:
        nc_new = _build_bass(t_steps)
        # finalize BEFORE handing to the PJRT path: the bass_exec lowering
        # otherwise finalizes with the partition-id register preamble in a
        # state that miscompiles (walrus "Reg has not been allocated yet")
        nc_new.finalize()
        _CACHED[key] = nc_new
    nc = _CACHED[key]
    in_maps = _prep_inputs(**inputs, t_steps=t_steps)
    res = None
    for attempt in range(4):
        try:
            res = run_bass_kernel_spmd(nc, in_maps, core_ids=list(range(NCORES)),
                                       trace=trace)
            break
        except Exception as e:  # flaky parallel-birverifier race in neuronx-cc
            if attempt == 3:
                raise
            print(f"run attempt {attempt} failed ({type(e).__name__}); retrying")
    assert res is not None
    y = np.concatenate([r["y"] for r in res.results], axis=0)
    return y, res


def kernel(**inputs):
    y, _ = run(t_steps=T, trace=False, **inputs)
    return y



# revision 39
# speedup vs baseline: 1.0016x; 1.0012x over previous
"""Trainium2 Bass kernel for a 2-layer LSTM (B=256, T=512, I=64, H=256) + linear head.

Strategy (hardcoded, self-contained):
  - Data-parallel over batch across 8 NeuronCores (32 batch elems per core).
  - Per core, both LSTM layers run step-by-step in a feature-blocked layout:
      gate PSUM tile [128=(hblk4, b32), 256=(gate4, hh2, hl32)]
    produced by col-group-packed bf16 matmuls (tile_position=(0, 32*m)) that
    share the small transposed-state stationary hT [k, 32]. bf16 operands
    avoid the fp32 LOW/HIGH double-pumped matmul passes (2x PE stream time);
    PSUM accumulation stays fp32 and the elementwise cell math stays fp32.
  - Input projection x@Wih.T and all biases ride the same PSUM accumulation
    (augmented ones-row trick), so there is no separate projection pass.
  - The kernel is recurrence-latency-bound: the wall time is the per-step
    dependency cycle mm -> sigmoid -> c-update -> tanh -> h -> transpose ->
    mm, with the two layers' chains interleaving at a half-round phase
    offset on the shared engines. Cell math is structured to shorten that
    cycle:
      * one merged Sigmoid over all four gates (g-gate weights are
        pre-scaled by 2 host-side; tanh(x) = 2*sigmoid(2x) - 1),
      * cell state tracked as c' = c/2 so the candidate term is ONE fused
        scalar_tensor_tensor op (sig(i)*(sig(g2)-0.5)) and tanh(c) is read
        back with the activation's free input scale (tanh(2*c')),
      * sig(f)*c' runs on GPSIMD off the critical path,
      * tanh writes its result to a PSUM bank (ScalarE sits closer to PSUM
        than to SBUF; measured ~3% whole-kernel win).
  - The two output linear layers have no nonlinearity between them and are
    folded host-side into a single [256, 4] matmul + bias.
  - All weights ship as ONE packed DRAM blob -> one DMA -> one HWDGE queue
    semaphore, keeping per-instruction sync-wait counts within HW limits.
"""

import os

import numpy as np

B, T, I, H, O = 256, 512, 64, 256, 4
NCORES = 8
BS = B // NCORES  # 32

# reference gate order is (i, f, g, o); we reorder to (i, f, o, g) so that the
# sigmoid gates are contiguous (cols 0:192) and tanh(g) is cols 192:256.
GATE_PERM = [0, 1, 3, 2]

# weight blob column offsets (fp32 elements, [128, WB_COLS])
OFF_W0 = 0        # Whh0 perm  [128, 2*1024]
OFF_W1 = 2048     # Whh1 perm  [128, 2*1024]
OFF_WX1 = 4096    # Wih1 perm  [128, 2*1024]
OFF_WX0 = 6144    # Wih0 perm + bias row, rows 0:65, [65, 1024]
OFF_B1 = 7168     # bias1 row, row 0, [1, 1024]
OFF_WF = 8192     # folded head weight [128, 2*4]
OFF_BF = 8200     # folded head bias, row 0, [1, 4]
OFF_XT = 8224     # x transposed + ones row, rows 0:65, [65, t_steps*32]
def _wb_cols(t_steps):
    return OFF_XT + t_steps * BS

_CACHED = {}


def _perm_cols(Wt):
    """Permute gate columns of [K, 1024] (col j = gate_orig*256 + h) into
    col = m*256 + gate_new*64 + hh*32 + hl, where h = hh*128 + m*32 + hl."""
    K = Wt.shape[0]
    W = Wt.reshape(K, 4, 256)[:, GATE_PERM, :]      # [K, gate, h]
    W = W.reshape(K, 4, 2, 4, 32)                    # [K, gate, hh, m, hl]
    W = W.transpose(0, 3, 1, 2, 4)                   # [K, m, gate, hh, hl]
    return np.ascontiguousarray(W.reshape(K, 1024), dtype=np.float32)


def _build_bass(t_steps=T):
    import concourse.mybir as mybir
    import concourse.tile as tile
    from concourse import bacc
    from contextlib import ExitStack

    f32 = mybir.dt.float32
    bf16 = mybir.dt.bfloat16
    AF = mybir.ActivationFunctionType

    nc = bacc.Bacc("TRN2", target_bir_lowering=False)

    wb_cols = _wb_cols(t_steps)
    wb_d = nc.dram_tensor("wb", (128, wb_cols), bf16, kind="ExternalInput")
    y_d = nc.dram_tensor("y", (BS, O), f32, kind="ExternalOutput")

    with tile.TileContext(nc) as tc, ExitStack() as ctx:
        const = ctx.enter_context(tc.tile_pool(name="const", bufs=1))
        state = ctx.enter_context(tc.tile_pool(name="state", bufs=1))
        work = ctx.enter_context(tc.tile_pool(name="work", bufs=6))
        hts = ctx.enter_context(tc.tile_pool(name="hts", bufs=4))
        psum = ctx.enter_context(tc.tile_pool(
            name="psum", bufs=int(os.environ.get("LSTM_GBUFS", "2")), space="PSUM"))

        wb = const.tile([128, wb_cols], bf16)
        nc.sync.dma_start(wb[:], wb_d[:])

        def xt_ap(t):
            return wb[0:65, OFF_XT + BS * t : OFF_XT + BS * t + BS]

        def w0_ap(kc, m):
            return wb[:, OFF_W0 + 1024 * kc + 256 * m : OFF_W0 + 1024 * kc + 256 * m + 256]

        def w1_ap(kc, m):
            return wb[:, OFF_W1 + 1024 * kc + 256 * m : OFF_W1 + 1024 * kc + 256 * m + 256]

        def wx1_ap(kc, m):
            return wb[:, OFF_WX1 + 1024 * kc + 256 * m : OFF_WX1 + 1024 * kc + 256 * m + 256]

        def wx0_ap(m):
            return wb[0:65, OFF_WX0 + 256 * m : OFF_WX0 + 256 * m + 256]

        def b1_ap(m):
            return wb[0:1, OFF_B1 + 256 * m : OFF_B1 + 256 * m + 256]

        c_psum = os.environ.get("LSTM_C_PSUM", "0") != "0"
        if c_psum:
            statep = ctx.enter_context(tc.tile_pool(name="statep", bufs=1, space="PSUM"))
            c0 = statep.tile([128, 64], f32, tag="c0")
            c1 = statep.tile([128, 64], f32, tag="c1")
        else:
            c0 = state.tile([128, 64], f32)
            c1 = state.tile([128, 64], f32)
        nc.vector.memset(c0[:], 0.0)
        nc.vector.memset(c1[:], 0.0)
        hT0 = hts.tile([128, 64], bf16, tag="ht0")
        hT1 = hts.tile([128, 64], bf16, tag="ht1")
        nc.vector.memset(hT0[:], 0.0)
        nc.vector.memset(hT1[:], 0.0)
        ones_t = const.tile([1, BS], bf16)
        nc.vector.memset(ones_t[:], 1.0)
        ones_ap = ones_t[:]

        from concourse.alu_op_type import AluOpType

        n_junk = int(os.environ.get("LSTM_JUNK", "0"))
        junkp = None
        if n_junk:
            junkp = ctx.enter_context(tc.tile_pool(name="junkp", bufs=2, space="PSUM"))

        def mms0(t, hT0_prev):
            g = psum.tile([128, 256], f32, tag="g0")
            for m in range(4):
                nc.tensor.matmul(
                    g[32 * m : 32 * m + 32, :], xt_ap(t), wx0_ap(m),
                    start=True, stop=False, tile_position=(0, 32 * m), skip_group_check=True,
                )
            for kc in range(2):
                for m in range(4):
                    nc.tensor.matmul(
                        g[32 * m : 32 * m + 32, :],
                        hT0_prev[:, 32 * kc : 32 * kc + 32], w0_ap(kc, m),
                        start=False, stop=(kc == 1), tile_position=(0, 32 * m), skip_group_check=True,
                    )
            return g

        def mms1(hT0_t, hT1_prev):
            g = psum.tile([128, 256], f32, tag="g1")
            for m in range(4):
                nc.tensor.matmul(
                    g[32 * m : 32 * m + 32, :], ones_ap, b1_ap(m),
                    start=True, stop=False, tile_position=(0, 32 * m), skip_group_check=True,
                )
            for src, w_ap in ((hT0_t, wx1_ap), (hT1_prev, w1_ap)):
                last_src = w_ap is w1_ap
                for kc in range(2):
                    for m in range(4):
                        nc.tensor.matmul(
                            g[32 * m : 32 * m + 32, :],
                            src[:, 32 * kc : 32 * kc + 32], w_ap(kc, m),
                            start=False, stop=(last_src and kc == 1),
                            tile_position=(0, 32 * m), skip_group_check=True,
                        )
            return g

        def junk_mms(n):
            # keep-warm matmuls: the PE HAM clock-gate only releases the
            # 2.4 GHz clock when the array looks busy; the real mm stream
            # alone leaves it at 1.2 GHz with micro-idles. Short N=128
            # streams into a scratch psum bank bound the queue-blocking cost.
            for _ in range(n):
                jt = junkp.tile([32, 128], f32, tag="junk")
                nc.tensor.matmul(jt[:], wb[:, 0:32], wb[:, 0:128],
                                 start=True, stop=True)

        def sig(g, tagsuf):
            # g-gate pre-activations arrive pre-scaled by 2 (host-side weight
            # prescale), so ONE Sigmoid covers all four gates:
            # tanh(x) = 2*sigmoid(2x) - 1.
            sg = work.tile([128, 256], f32, tag="sg" + tagsuf)
            nc.scalar.activation(sg[:], g[:], AF.Sigmoid)
            return sg

        def cphase(sg, c, tagsuf):
            # state is tracked as c' = c/2:
            #   tanh(g-pre) = 2*sig(g2) - 1, so
            #   m1' = sig(i)*tanh(g)/2 = sig(i)*(sig(g2) - 0.5)   [one fused STT]
            #   c'  = sig(f)*c' + m1'
            m1 = work.tile([128, 64], f32, tag="m1" + tagsuf)
            if c_psum:
                # GPSIMD cannot target PSUM; give it the SBUF-only STT and
                # keep the c ops (PSUM-resident) on the DVE.
                nc.gpsimd.scalar_tensor_tensor(
                    m1[:], sg[:, 192:256], 0.5, sg[:, 0:64],
                    AluOpType.subtract, AluOpType.mult)
                nc.vector.tensor_mul(c[:], sg[:, 64:128], c[:])
            else:
                nc.vector.scalar_tensor_tensor(
                    m1[:], sg[:, 192:256], 0.5, sg[:, 0:64],
                    AluOpType.subtract, AluOpType.mult)
                nc.gpsimd.tensor_mul(c[:], sg[:, 64:128], c[:])
            nc.vector.tensor_add(c[:], c[:], m1[:])

        th_psum = os.environ.get("LSTM_TH_PSUM", "1") != "0"
        thp = None
        th_all = None
        if c_psum:
            # both chains' th in ONE persistent psum tile (bank budget)
            th_all = statep.tile([128, 128], f32, tag="th")
        elif th_psum:
            thp = ctx.enter_context(tc.tile_pool(name="thp", bufs=1, space="PSUM"))

        def tanph(c, tagsuf):
            # tanh(c) = tanh(2*c') via the free input affine
            if c_psum:
                th = th_all[:, 0:64] if tagsuf == "0" else th_all[:, 64:128]
                nc.scalar.activation(th, c[:], AF.Tanh, scale=2.0)
                return th
            pool = thp if th_psum else work
            th = pool.tile([128, 64], f32, tag="th" + tagsuf)
            nc.scalar.activation(th[:], c[:], AF.Tanh, scale=2.0)
            return th

        split_h = os.environ.get("LSTM_SPLIT_H", "0") != "0"

        def hphase(sg, th, tagsuf):
            # split h = sig(o)*th and the 32x32-block transpose into column
            # halves: the next step's kc0 matmul only needs hT[:, 0:32], so it
            # can start while the second half is still on the DVE.
            h = work.tile([128, 64], bf16, tag="h" + tagsuf)
            hT = hts.tile([128, 64], bf16, tag="ht" + tagsuf)
            if split_h:
                for half in range(2):
                    s = slice(32 * half, 32 * half + 32)
                    nc.vector.tensor_mul(h[:, s], sg[:, 128 + 32 * half : 160 + 32 * half], th[:, s])
                    nc.vector.transpose(hT[:, s], h[:, s])
            else:
                nc.vector.tensor_mul(h[:], sg[:, 128:192], th[:])
                nc.vector.transpose(hT[:], h[:])
            return hT

        # rounds: round t runs layer0 step t (A) and layer1 step t-1 (B).
        # In the half-offset steady state the per-engine readiness order is
        # full-A-chain then full-B-chain, so emit whole chains sequentially —
        # interleaving phases head-of-line-blocks the FIFO engine queues.
        hT0_prev, hT1_prev = hT0, hT1
        # Emission anchors the B chain a quarter-round behind A: queueing
        # sigB right after sigA on the ACT FIFO phase-locks the chains away
        # from the half-offset fixed point where B's tanh/transposes collide
        # with A's sigma/STT.
        # The two chains settle into a half-round phase offset; emitting each
        # chain's ops as a contiguous block in A-then-B order matches the
        # per-engine readiness order at that fixed point.
        for t in range(t_steps + 1):
            gA = mms0(t, hT0_prev) if t < t_steps else None
            if n_junk:
                junk_mms(n_junk // 2)
            gB = mms1(hT0_prev, hT1_prev) if t >= 1 else None
            if n_junk:
                junk_mms(n_junk - n_junk // 2)
            if gA is not None:
                sgA = sig(gA, "0")
                cphase(sgA, c0, "0")
                thA = tanph(c0, "0")
                hT0_new = hphase(sgA, thA, "0")
            if gB is not None:
                sgB = sig(gB, "1")
                cphase(sgB, c1, "1")
                thB = tanph(c1, "1")
                hT1_prev = hphase(sgB, thB, "1")
            if gA is not None:
                hT0_prev = hT0_new
        hT1 = hT1_prev

        yp = psum.tile([BS, O], f32, tag="yh", bufs=1)
        nc.tensor.matmul(yp[:], ones_ap, wb[0:1, OFF_BF : OFF_BF + O], start=True, stop=False)
        nc.tensor.matmul(yp[:], hT1[:, 0:32], wb[:, OFF_WF : OFF_WF + O], start=False, stop=False)
        nc.tensor.matmul(yp[:], hT1[:, 32:64], wb[:, OFF_WF + O : OFF_WF + 2 * O], start=False, stop=True)
        y_sb = work.tile([BS, O], f32, tag="y")
        nc.vector.tensor_copy(y_sb[:], yp[:])
        nc.sync.dma_start(y_d[:], y_sb[:])

    return nc


def _prep_inputs(x, Wih0, Whh0, bih0, bhh0, Wih1, Whh1, bih1, bhh1, W1, b1, W2, b2,
                 t_steps=T):
    x = np.asarray(x, dtype=np.float32)[:, :t_steps, :]

    def g2(Wt):
        # prescale the g-gate columns (original order i,f,g,o => cols 512:768)
        # by 2 so the kernel can compute tanh(x) as 2*sigmoid(2x)-1 with a
        # single Sigmoid activation over all four gates.
        Wt = np.asarray(Wt, np.float32).copy()
        Wt[..., 512:768] *= 2.0
        return Wt

    wb = np.zeros((128, _wb_cols(t_steps)), np.float32)
    wb[:, OFF_W0 : OFF_W0 + 2048] = _perm_cols(
        g2(np.asarray(Whh0, np.float32).T)).reshape(2, 128, 1024).transpose(1, 0, 2).reshape(128, 2048)
    wb[:, OFF_W1 : OFF_W1 + 2048] = _perm_cols(
        g2(np.asarray(Whh1, np.float32).T)).reshape(2, 128, 1024).transpose(1, 0, 2).reshape(128, 2048)
    wb[:, OFF_WX1 : OFF_WX1 + 2048] = _perm_cols(
        g2(np.asarray(Wih1, np.float32).T)).reshape(2, 128, 1024).transpose(1, 0, 2).reshape(128, 2048)
    wb[0:64, OFF_WX0 : OFF_WX0 + 1024] = _perm_cols(g2(np.asarray(Wih0, np.float32).T))
    wb[64, OFF_WX0 : OFF_WX0 + 1024] = _perm_cols(
        g2((np.asarray(bih0, np.float32) + np.asarray(bhh0, np.float32))[None, :]))[0]
    wb[0, OFF_B1 : OFF_B1 + 1024] = _perm_cols(
        g2((np.asarray(bih1, np.float32) + np.asarray(bhh1, np.float32))[None, :]))[0]
    Wf = (np.asarray(W1, np.float32).T @ np.asarray(W2, np.float32).T).astype(np.float32)
    wb[:, OFF_WF : OFF_WF + 2 * O] = Wf.reshape(2, 128, O).transpose(1, 0, 2).reshape(128, 2 * O)
    wb[0, OFF_BF : OFF_BF + O] = (
        np.asarray(b1, np.float32) @ np.asarray(W2, np.float32).T + np.asarray(b2, np.float32))

    import ml_dtypes

    in_maps = []
    for c in range(NCORES):
        xc = x[c * BS : (c + 1) * BS]                       # [BS, t, I]
        xt = xc.transpose(2, 1, 0).reshape(I, t_steps * BS) # [I, t*BS]
        wbc = wb.copy()
        wbc[0:64, OFF_XT:] = xt
        wbc[64, OFF_XT:] = 1.0
        in_maps.append(dict(wb=wbc.astype(ml_dtypes.bfloat16)))
    return in_maps


def run(t_steps=T, trace=False, **inputs):
    from concourse.bass_utils import run_bass_kernel_spmd

    key = t_steps
    if key not in _CACHED:
        nc_new = _build_bass(t_steps)
        # finalize BEFORE handing to the PJRT path: the bass_exec lowering
        # otherwise finalizes with the partition-id register preamble in a
        # state that miscompiles (walrus "Reg has not been allocated yet")
        nc_new.finalize()
        _CACHED[key] = nc_new
    nc = _CACHED[key]
    in_maps = _prep_inputs(**inputs, t_steps=t_steps)
    res = None
    for attempt in range(4):
        try:
            res = run_bass_kernel_spmd(nc, in_maps, core_ids=list(range(NCORES)),
                                       trace=trace)
            break
        except Exception as e:  # flaky parallel-birverifier race in neuronx-cc
            if attempt == 3:
                raise
            print(f"run attempt {attempt} failed ({type(e).__name__}); retrying")
    assert res is not None
    y = np.concatenate([r["y"] for r in res.results], axis=0)
    return y, res


def kernel(**inputs):
    y, _ = run(t_steps=T, trace=False, **inputs)
    return y



# revision 41
# speedup vs baseline: 1.0094x; 1.0078x over previous
"""Trainium2 Bass kernel for a 2-layer LSTM (B=256, T=512, I=64, H=256) + linear head.

Strategy (hardcoded, self-contained):
  - Data-parallel over batch across 8 NeuronCores (32 batch elems per core).
  - Per core, both LSTM layers run step-by-step in a feature-blocked layout:
      gate PSUM tile [128=(hblk4, b32), 256=(gate4, hh2, hl32)]
    produced by col-group-packed bf16 matmuls (tile_position=(0, 32*m)) that
    share the small transposed-state stationary hT [k, 32]. bf16 operands
    avoid the fp32 LOW/HIGH double-pumped matmul passes (2x PE stream time);
    PSUM accumulation stays fp32 and the elementwise cell math stays fp32.
  - Input projection x@Wih.T and all biases ride the same PSUM accumulation
    (augmented ones-row trick), so there is no separate projection pass.
  - The kernel is recurrence-latency-bound: the wall time is the per-step
    dependency cycle mm -> sigmoid -> c-update -> tanh -> h -> transpose ->
    mm, with the two layers' chains interleaving at a half-round phase
    offset on the shared engines. Cell math is structured to shorten that
    cycle:
      * one merged Sigmoid over all four gates (g-gate weights are
        pre-scaled by 2 host-side; tanh(x) = 2*sigmoid(2x) - 1),
      * cell state tracked as c' = c/2 so the candidate term is ONE fused
        scalar_tensor_tensor op (sig(i)*(sig(g2)-0.5)) and tanh(c) is read
        back with the activation's free input scale (tanh(2*c')),
      * sig(f)*c' runs on GPSIMD off the critical path,
      * tanh writes its result to a PSUM bank (ScalarE sits closer to PSUM
        than to SBUF; measured ~3% whole-kernel win).
  - The two output linear layers have no nonlinearity between them and are
    folded host-side into a single [256, 4] matmul + bias.
  - All weights ship as ONE packed DRAM blob -> one DMA -> one HWDGE queue
    semaphore, keeping per-instruction sync-wait counts within HW limits.
"""

import os

import numpy as np

B, T, I, H, O = 256, 512, 64, 256, 4
NCORES = 8
BS = B // NCORES  # 32

# reference gate order is (i, f, g, o); we reorder to (i, f, o, g) so that the
# sigmoid gates are contiguous (cols 0:192) and tanh(g) is cols 192:256.
GATE_PERM = [0, 1, 3, 2]

# weight blob column offsets (fp32 elements, [128, WB_COLS])
OFF_W0 = 0        # Whh0 perm  [128, 2*1024]
OFF_W1 = 2048     # Whh1 perm  [128, 2*1024]
OFF_WX1 = 4096    # Wih1 perm  [128, 2*1024]
OFF_WX0 = 6144    # Wih0 perm + bias row, rows 0:65, [65, 1024]
OFF_B1 = 7168     # bias1 row, row 0, [1, 1024]
OFF_WF = 8192     # folded head weight [128, 2*4]
OFF_BF = 8200     # folded head bias, row 0, [1, 4]
OFF_XT = 8224     # x transposed + ones row, rows 0:65, [65, t_steps*32]
def _wb_cols(t_steps):
    return OFF_XT + t_steps * BS

_CACHED = {}


def _perm_cols(Wt):
    """Permute gate columns of [K, 1024] (col j = gate_orig*256 + h) into
    col = m*256 + gate_new*64 + hh*32 + hl, where h = hh*128 + m*32 + hl."""
    K = Wt.shape[0]
    W = Wt.reshape(K, 4, 256)[:, GATE_PERM, :]      # [K, gate, h]
    W = W.reshape(K, 4, 2, 4, 32)                    # [K, gate, hh, m, hl]
    W = W.transpose(0, 3, 1, 2, 4)                   # [K, m, gate, hh, hl]
    return np.ascontiguousarray(W.reshape(K, 1024), dtype=np.float32)


def _build_bass(t_steps=T):
    import concourse.mybir as mybir
    import concourse.tile as tile
    from concourse import bacc
    from contextlib import ExitStack

    f32 = mybir.dt.float32
    bf16 = mybir.dt.bfloat16
    AF = mybir.ActivationFunctionType

    nc = bacc.Bacc("TRN2", target_bir_lowering=False)

    wb_cols = _wb_cols(t_steps)
    wb_d = nc.dram_tensor("wb", (128, wb_cols), bf16, kind="ExternalInput")
    y_d = nc.dram_tensor("y", (BS, O), f32, kind="ExternalOutput")

    with tile.TileContext(nc) as tc, ExitStack() as ctx:
        const = ctx.enter_context(tc.tile_pool(name="const", bufs=1))
        state = ctx.enter_context(tc.tile_pool(name="state", bufs=1))
        work = ctx.enter_context(tc.tile_pool(name="work", bufs=6))
        hts = ctx.enter_context(tc.tile_pool(name="hts", bufs=4))
        psum = ctx.enter_context(tc.tile_pool(
            name="psum", bufs=int(os.environ.get("LSTM_GBUFS", "2")), space="PSUM"))

        wb = const.tile([128, wb_cols], bf16)
        nc.sync.dma_start(wb[:], wb_d[:])

        def xt_ap(t):
            return wb[0:65, OFF_XT + BS * t : OFF_XT + BS * t + BS]

        def w0_ap(kc, m):
            return wb[:, OFF_W0 + 1024 * kc + 256 * m : OFF_W0 + 1024 * kc + 256 * m + 256]

        def w1_ap(kc, m):
            return wb[:, OFF_W1 + 1024 * kc + 256 * m : OFF_W1 + 1024 * kc + 256 * m + 256]

        def wx1_ap(kc, m):
            return wb[:, OFF_WX1 + 1024 * kc + 256 * m : OFF_WX1 + 1024 * kc + 256 * m + 256]

        def wx0_ap(m):
            return wb[0:65, OFF_WX0 + 256 * m : OFF_WX0 + 256 * m + 256]

        def b1_ap(m):
            return wb[0:1, OFF_B1 + 256 * m : OFF_B1 + 256 * m + 256]

        c_psum = os.environ.get("LSTM_C_PSUM", "0") != "0"
        if c_psum:
            statep = ctx.enter_context(tc.tile_pool(name="statep", bufs=1, space="PSUM"))
            c0 = statep.tile([128, 64], f32, tag="c0")
            c1 = statep.tile([128, 64], f32, tag="c1")
        else:
            c0 = state.tile([128, 64], f32)
            c1 = state.tile([128, 64], f32)
        nc.vector.memset(c0[:], 0.0)
        nc.vector.memset(c1[:], 0.0)
        hT0 = hts.tile([128, 64], bf16, tag="ht0")
        hT1 = hts.tile([128, 64], bf16, tag="ht1")
        nc.vector.memset(hT0[:], 0.0)
        nc.vector.memset(hT1[:], 0.0)
        ones_t = const.tile([1, BS], bf16)
        nc.vector.memset(ones_t[:], 1.0)
        ones_ap = ones_t[:]

        from concourse.alu_op_type import AluOpType

        n_junk = int(os.environ.get("LSTM_JUNK", "0"))
        junkp = None
        if n_junk:
            junkp = ctx.enter_context(tc.tile_pool(name="junkp", bufs=2, space="PSUM"))

        def mms0(t, hT0_prev):
            g = psum.tile([128, 256], f32, tag="g0")
            for m in range(4):
                nc.tensor.matmul(
                    g[32 * m : 32 * m + 32, :], xt_ap(t), wx0_ap(m),
                    start=True, stop=False, tile_position=(0, 32 * m), skip_group_check=True,
                )
            for kc in range(2):
                for m in range(4):
                    nc.tensor.matmul(
                        g[32 * m : 32 * m + 32, :],
                        hT0_prev[:, 32 * kc : 32 * kc + 32], w0_ap(kc, m),
                        start=False, stop=(kc == 1), tile_position=(0, 32 * m), skip_group_check=True,
                    )
            return g

        def mms1(hT0_t, hT1_prev):
            g = psum.tile([128, 256], f32, tag="g1")
            for m in range(4):
                nc.tensor.matmul(
                    g[32 * m : 32 * m + 32, :], ones_ap, b1_ap(m),
                    start=True, stop=False, tile_position=(0, 32 * m), skip_group_check=True,
                )
            for src, w_ap in ((hT0_t, wx1_ap), (hT1_prev, w1_ap)):
                last_src = w_ap is w1_ap
                for kc in range(2):
                    for m in range(4):
                        nc.tensor.matmul(
                            g[32 * m : 32 * m + 32, :],
                            src[:, 32 * kc : 32 * kc + 32], w_ap(kc, m),
                            start=False, stop=(last_src and kc == 1),
                            tile_position=(0, 32 * m), skip_group_check=True,
                        )
            return g

        def junk_mms(n):
            # keep-warm matmuls: the PE HAM clock-gate only releases the
            # 2.4 GHz clock when the array looks busy; the real mm stream
            # alone leaves it at 1.2 GHz with micro-idles. Short N=128
            # streams into a scratch psum bank bound the queue-blocking cost.
            for _ in range(n):
                jt = junkp.tile([32, 128], f32, tag="junk")
                nc.tensor.matmul(jt[:], wb[:, 0:32], wb[:, 0:128],
                                 start=True, stop=True)

        sg_bf16 = os.environ.get("LSTM_SG_BF16", "0") != "0"
        sg_dt = bf16 if sg_bf16 else f32

        def sig(g, tagsuf):
            # g-gate pre-activations arrive pre-scaled by 2 (host-side weight
            # prescale), so ONE Sigmoid covers all four gates:
            # tanh(x) = 2*sigmoid(2x) - 1.
            sg = work.tile([128, 256], sg_dt, tag="sg" + tagsuf)
            nc.scalar.activation(sg[:], g[:], AF.Sigmoid)
            return sg

        def cphase(sg, c, tagsuf):
            # state is tracked as c' = c/2:
            #   tanh(g-pre) = 2*sig(g2) - 1, so
            #   m1' = sig(i)*tanh(g)/2 = sig(i)*(sig(g2) - 0.5)   [one fused STT]
            #   c'  = sig(f)*c' + m1'
            m1 = work.tile([128, 64], sg_dt, tag="m1" + tagsuf)
            if c_psum:
                # GPSIMD cannot target PSUM; give it the SBUF-only STT and
                # keep the c ops (PSUM-resident) on the DVE.
                nc.gpsimd.scalar_tensor_tensor(
                    m1[:], sg[:, 192:256], 0.5, sg[:, 0:64],
                    AluOpType.subtract, AluOpType.mult)
                nc.vector.tensor_mul(c[:], sg[:, 64:128], c[:])
            else:
                nc.vector.scalar_tensor_tensor(
                    m1[:], sg[:, 192:256], 0.5, sg[:, 0:64],
                    AluOpType.subtract, AluOpType.mult)
                nc.gpsimd.tensor_mul(c[:], sg[:, 64:128], c[:])
            nc.vector.tensor_add(c[:], c[:], m1[:])

        th_psum = os.environ.get("LSTM_TH_PSUM", "1") != "0"
        thp = None
        th_all = None
        if c_psum:
            # both chains' th in ONE persistent psum tile (bank budget)
            th_all = statep.tile([128, 128], f32, tag="th")
        elif th_psum:
            thp = ctx.enter_context(tc.tile_pool(name="thp", bufs=1, space="PSUM"))

        def tanph(c, tagsuf):
            # tanh(c) = tanh(2*c') via the free input affine
            if c_psum:
                th = th_all[:, 0:64] if tagsuf == "0" else th_all[:, 64:128]
                nc.scalar.activation(th, c[:], AF.Tanh, scale=2.0)
                return th
            pool = thp if th_psum else work
            th = pool.tile([128, 64], f32, tag="th" + tagsuf)
            nc.scalar.activation(th[:], c[:], AF.Tanh, scale=2.0)
            return th

        split_h = os.environ.get("LSTM_SPLIT_H", "0") != "0"

        def hphase(sg, th, tagsuf):
            # split h = sig(o)*th and the 32x32-block transpose into column
            # halves: the next step's kc0 matmul only needs hT[:, 0:32], so it
            # can start while the second half is still on the DVE.
            h = work.tile([128, 64], bf16, tag="h" + tagsuf)
            hT = hts.tile([128, 64], bf16, tag="ht" + tagsuf)
            if split_h:
                for half in range(2):
                    s = slice(32 * half, 32 * half + 32)
                    nc.vector.tensor_mul(h[:, s], sg[:, 128 + 32 * half : 160 + 32 * half], th[:, s])
                    nc.vector.transpose(hT[:, s], h[:, s])
            else:
                nc.vector.tensor_mul(h[:], sg[:, 128:192], th[:])
                nc.vector.transpose(hT[:], h[:])
            return hT

        # rounds: round t runs layer0 step t (A) and layer1 step t-1 (B).
        # In the half-offset steady state the per-engine readiness order is
        # full-A-chain then full-B-chain, so emit whole chains sequentially —
        # interleaving phases head-of-line-blocks the FIFO engine queues.
        hT0_prev, hT1_prev = hT0, hT1
        # Emission anchors the B chain a quarter-round behind A: queueing
        # sigB right after sigA on the ACT FIFO phase-locks the chains away
        # from the half-offset fixed point where B's tanh/transposes collide
        # with A's sigma/STT.
        # The two chains settle into a half-round phase offset; emitting each
        # chain's ops as a contiguous block in A-then-B order matches the
        # per-engine readiness order at that fixed point.
        for t in range(t_steps + 1):
            gA = mms0(t, hT0_prev) if t < t_steps else None
            if n_junk:
                junk_mms(n_junk // 2)
            gB = mms1(hT0_prev, hT1_prev) if t >= 1 else None
            if n_junk:
                junk_mms(n_junk - n_junk // 2)
            if gA is not None:
                sgA = sig(gA, "0")
                cphase(sgA, c0, "0")
                thA = tanph(c0, "0")
                hT0_new = hphase(sgA, thA, "0")
            if gB is not None:
                sgB = sig(gB, "1")
                cphase(sgB, c1, "1")
                thB = tanph(c1, "1")
                hT1_prev = hphase(sgB, thB, "1")
            if gA is not None:
                hT0_prev = hT0_new
        hT1 = hT1_prev

        yp = psum.tile([BS, O], f32, tag="yh", bufs=1)
        nc.tensor.matmul(yp[:], ones_ap, wb[0:1, OFF_BF : OFF_BF + O], start=True, stop=False)
        nc.tensor.matmul(yp[:], hT1[:, 0:32], wb[:, OFF_WF : OFF_WF + O], start=False, stop=False)
        nc.tensor.matmul(yp[:], hT1[:, 32:64], wb[:, OFF_WF + O : OFF_WF + 2 * O], start=False, stop=True)
        y_sb = work.tile([BS, O], f32, tag="y")
        nc.vector.tensor_copy(y_sb[:], yp[:])
        nc.sync.dma_start(y_d[:], y_sb[:])

    return nc


def _prep_inputs(x, Wih0, Whh0, bih0, bhh0, Wih1, Whh1, bih1, bhh1, W1, b1, W2, b2,
                 t_steps=T):
    x = np.asarray(x, dtype=np.float32)[:, :t_steps, :]

    def g2(Wt):
        # prescale the g-gate columns (original order i,f,g,o => cols 512:768)
        # by 2 so the kernel can compute tanh(x) as 2*sigmoid(2x)-1 with a
        # single Sigmoid activation over all four gates.
        Wt = np.asarray(Wt, np.float32).copy()
        Wt[..., 512:768] *= 2.0
        return Wt

    wb = np.zeros((128, _wb_cols(t_steps)), np.float32)
    wb[:, OFF_W0 : OFF_W0 + 2048] = _perm_cols(
        g2(np.asarray(Whh0, np.float32).T)).reshape(2, 128, 1024).transpose(1, 0, 2).reshape(128, 2048)
    wb[:, OFF_W1 : OFF_W1 + 2048] = _perm_cols(
        g2(np.asarray(Whh1, np.float32).T)).reshape(2, 128, 1024).transpose(1, 0, 2).reshape(128, 2048)
    wb[:, OFF_WX1 : OFF_WX1 + 2048] = _perm_cols(
        g2(np.asarray(Wih1, np.float32).T)).reshape(2, 128, 1024).transpose(1, 0, 2).reshape(128, 2048)
    wb[0:64, OFF_WX0 : OFF_WX0 + 1024] = _perm_cols(g2(np.asarray(Wih0, np.float32).T))
    wb[64, OFF_WX0 : OFF_WX0 + 1024] = _perm_cols(
        g2((np.asarray(bih0, np.float32) + np.asarray(bhh0, np.float32))[None, :]))[0]
    wb[0, OFF_B1 : OFF_B1 + 1024] = _perm_cols(
        g2((np.asarray(bih1, np.float32) + np.asarray(bhh1, np.float32))[None, :]))[0]
    Wf = (np.asarray(W1, np.float32).T @ np.asarray(W2, np.float32).T).astype(np.float32)
    wb[:, OFF_WF : OFF_WF + 2 * O] = Wf.reshape(2, 128, O).transpose(1, 0, 2).reshape(128, 2 * O)
    wb[0, OFF_BF : OFF_BF + O] = (
        np.asarray(b1, np.float32) @ np.asarray(W2, np.float32).T + np.asarray(b2, np.float32))

    import ml_dtypes

    in_maps = []
    for c in range(NCORES):
        xc = x[c * BS : (c + 1) * BS]                       # [BS, t, I]
        xt = xc.transpose(2, 1, 0).reshape(I, t_steps * BS) # [I, t*BS]
        wbc = wb.copy()
        wbc[0:64, OFF_XT:] = xt
        wbc[64, OFF_XT:] = 1.0
        in_maps.append(dict(wb=wbc.astype(ml_dtypes.bfloat16)))
    return in_maps


def run(t_steps=T, trace=False, **inputs):
    from concourse.bass_utils import run_bass_kernel_spmd

    key = t_steps
    if key not in _CACHED:
        nc_new = _build_bass(t_steps)
        # finalize BEFORE handing to the PJRT path: the bass_exec lowering
        # otherwise finalizes with the partition-id register preamble in a
        # state that miscompiles (walrus "Reg has not been allocated yet")
        nc_new.finalize()
        _CACHED[key] = nc_new
    nc = _CACHED[key]
    in_maps = _prep_inputs(**inputs, t_steps=t_steps)
    res = None
    for attempt in range(4):
        try:
            res = run_bass_kernel_spmd(nc, in_maps, core_ids=list(range(NCORES)),
                                       trace=trace)
            break
        except Exception as e:  # flaky parallel-birverifier race in neuronx-cc
            if attempt == 3:
                raise
            print(f"run attempt {attempt} failed ({type(e).__name__}); retrying")
    assert res is not None
    y = np.concatenate([r["y"] for r in res.results], axis=0)
    return y, res


def kernel(**inputs):
    y, _ = run(t_steps=T, trace=False, **inputs)
    return y

